# revision 73
# baseline (speedup 1.0000x reference)
"""Trainium2 Bass kernel for nn_Attention_78786880078481.

Full (unsharded) inputs in, full output out. Sharding: data-parallel over the
batch dim (B=8) across the 8 NeuronCores - one batch element per core, no
collectives needed.

v5: single packed-fp16 blob input + fully fused pipeline (sim 191us/core vs
233us for the fp32r two-window baseline).

All device data is fp16 (the rel-err budget is 2e-2; fp16 end-to-end costs
~6e-4).  The host packs ONE blob tensor per core:
    [xT | qT(half-major) | Wk(strips) | Wv | Wq(strips) | Wp | consts]
([6277, 1024] fp16, ~12.9MB) with x and query pre-transposed on the host, so
the kernel needs no PE transposes at all.  A single input tensor also
minimizes the per-call input staging cost of the runtime (measured ~55us per
extra input binding plus ~3.4us/MB on this stack), which dominates the
end-to-end marginal time.  Wk/Wq are packed strip-major (one 128-row blob
block per 128-wide output-column strip) and qT half-major, so the DMA stream
delivers exactly the bytes scores(0) needs first: the exp stream starts
~18us in instead of ~34us.

Per-core computation (S=1024, NX=1024, H=16, HD=64), fp16 matmuls (full PE
rate at any moving width - no fp32r >=256-wide constraint, so causal score /
attnT blocks are exact-width), fp32 PSUM accumulation:

  Prologue: DMA stream [x+wv chunks | wk strip 0 | wq strip 0 | qT t0-half |
     remaining strips | qT t1-half | wp] on both HWDGE rings in consumption
     order (the transfer engine is a single ~330GB/s resource).  Vq0 and
     kproj(0) run k-major with six parallel PSUM accumulation groups, paced
     by the chunk stream; then Vq0b, qproj(t0,0), scores(0) at ~18us.
  Fused t0 pass (sq 0..511), unit m = heads (2m, 2m+1):
     scores(t0,m) -> exp on the scalar engine (odd head first so its exp
     stream drains under the even head's score matmuls); PE filler between
     scores and attnT hides the exp latency: kproj(m+1) just-in-time for
     the next unit, one Vproj quarter-half (m<=5), qproj(t0,m+1);
     then attnT(t0,m) + normalize (DVE reciprocal + gpsimd partition
     broadcast; odd head's rows reach partition offset 64 via SBUF-SBUF
     DMA).  Masked softmax needs no max-subtraction: |scores/8| < ~2 and
     the reference's -1e4 bias underflows to exact 0 after its own
     max-subtraction, matching our never-computed masked blocks.
  t1 pass (sq 512..1023): scores (block-paired exp instructions; the
     below-diagonal stale-PSUM gap regions are bounded so exp cannot
     overflow), one c_proj group of the finished t0 rows per unit, plus
     qproj(t1,m+1) as PE filler so the scalar engine (8us of exp per unit)
     never becomes the unit bottleneck, then attnT.
  Trail: c_proj for the t1 rows, software-pipelined two groups deep
     (accumulators alternate between the pd and sc1 PSUM rings): each
     group's k=7 finisher - the only part that waits on unit 7's
     normalize - trails its k=0..6 accumulation by two groups, and the
     first two accumulations are interleaved into unit 7 itself.  Each
     row block's two halves stage into one [128, NX] tile (the two
     PSUM-drain copies on DVE and Act in parallel) and ship as a single
     merged DMA, halving the epilogue's descriptor-generator load so the
     end-of-kernel barrier is not gated on a drained-out DMA queue.

Output is fp16 on device (halves the out staging + DMA; out DMAs alternate
rings, yo ring 6 deep so the epilogue is not copy->DMA serialized), upcast
to fp32 on the host.  Biases are zeros in setup_inputs(); bias matmuls (via
a ones-row outer product from the blob's bias rows) are emitted only if
nonzero.
"""

import sys

for p in ("/opt/trn_rl_repo",):
    if p not in sys.path:
        sys.path.insert(0, p)

import numpy as np

import concourse.bass as bass
import concourse.tile as tile
from concourse import bacc, mybir
from concourse.bass_utils import run_bass_kernel_spmd

F32 = mybir.dt.float32
F16 = mybir.dt.float16
EXPF = mybir.ActivationFunctionType.Exp

_CACHE = {}
BUILD_MARKS = []  # (label, n_instructions) snapshots for profiling tools

P = 128


def build_module(S, NX, H, with_attn_bias, with_proj_bias, n_cores=8):
    """Build the per-core Bass module."""
    from contextlib import ExitStack

    HD = NX // H
    assert HD == 64, "kernel specialized for head_dim 64 (2 heads per 128-row chunk)"
    SB = S // P        # number of 128-row blocks of S
    KB = NX // P       # number of 128-deep contraction chunks over NX
    CW = min(512, S)   # column-tile width over S
    assert S // CW == 2 and CW == 512, "schedule specialized for S=1024"
    NW = 256           # Vproj quarter width (2 heads)
    HC = HD + 1        # head stride in v_aug (v columns + ones column)
    scale = 1.0 / float(np.sqrt(HD))
    wab, wpb = with_attn_bias, with_proj_bias

    # blob row regions
    R_XT, R_QT, R_WK, R_WV, R_WQ, R_WP = 0, NX, 2 * NX, 3 * NX, 4 * NX, 5 * NX
    R_CONST = 6 * NX          # P rows: cols [0,P) mask, [P,P+H) ones
    R_BIAS = 6 * NX + P       # 5 rows: ones-row, bq, bk, bv, bp
    R_TOT = R_BIAS + 5

    nc = bacc.Bacc("TRN2", target_bir_lowering=False, debug=False,
                   num_devices=n_cores)

    blob_d = nc.dram_tensor("blob", [R_TOT, NX], F16, kind="ExternalInput")
    out_d = nc.dram_tensor("out", [S, NX], F16, kind="ExternalOutput")

    BUILD_MARKS.clear()

    def mark(label):
        BUILD_MARKS.append((label, len(nc.inst_map)))

    with tile.TileContext(nc) as tc, ExitStack() as top:
        consts = top.enter_context(tc.tile_pool(name="consts", bufs=1))
        cm = consts.tile([P, P + H], F16, tag="cm")
        mask = cm[:, :P]          # mask[sk, sq] = 1 where sk <= sq
        ones16 = cm[:, P : P + H]
        nc.scalar.dma_start(cm[:], blob_d[R_CONST : R_CONST + P, : P + H])
        if wab or wpb:
            # all five rows land on partition 0 (matmul operands must have
            # base partition 0/32/64), side by side in columns
            bias = consts.tile([1, 5 * NX], F16, tag="bias")
            nc.scalar.dma_start(
                bias.rearrange("p (r c) -> p r c", c=NX),
                blob_d[R_BIAS : R_BIAS + 5, :].rearrange("(p r) c -> p r c", p=1),
            )
            ones_row, b_q, b_k, b_v, b_p = (
                bias[0:1, i * NX : (i + 1) * NX] for i in range(5)
            )

        persist = top.enter_context(tc.tile_pool(name="persist", bufs=1))
        kT = persist.tile([P, KB * S], F16, tag="kT")        # [NX, S]
        v_aug = persist.tile([P, SB * H * HC], F16, tag="v")
        qT1 = persist.tile([P, SB * CW], F16, tag="qT1")     # t1 q-stash
        wp = persist.tile([P, KB * NX], F16, tag="wp")

        aT_pool = top.enter_context(tc.tile_pool(name="aTp", bufs=1))
        # one tile per (k-chunk, sq-half): exact read/write dependencies so
        # c_proj accumulation never waits on an unrelated chunk's normalize
        aTt = [
            [
                aT_pool.tile([P, CW], F16, tag=f"aT{k}_{t}", name=f"aT{k}_{t}")
                for t in range(2)
            ]
            for k in range(KB)
        ]
        nrm_pool = top.enter_context(tc.tile_pool(name="nrm", bufs=3))
        out_alt = [0]
        yob_cur = [None]

        def out_dma(dst, src):
            ring = nc.sync if out_alt[0] % 2 == 0 else nc.scalar
            out_alt[0] += 1
            ring.dma_start(dst, src)
        at_pool = top.enter_context(tc.tile_pool(name="at", bufs=2, space="PSUM"))

        # Resources released after the t0 pass live on the RIGHT side of the
        # heap so their (early) release keeps LIFO order on each side.
        # stq_b (qT/wq/qTmp) survives into the t1 pass; stq_a (xT/wk/wv,
        # pt0, PSUM rings) closes right after the t0 pass — allocated last
        # on the right side so its release keeps LIFO order.
        stq_b = ExitStack()
        bigb = stq_b.enter_context(tc.tile_pool(name="bigb", bufs=1, side="right"))
        qT = bigb.tile([P, KB * S], F16, tag="qT")
        wq = bigb.tile([P, KB * NX], F16, tag="wq")
        qTmp_pool = stq_b.enter_context(
            tc.tile_pool(name="qTmp", bufs=3, side="right")
        )
        stq = ExitStack()
        biga = stq.enter_context(tc.tile_pool(name="biga", bufs=1, side="right"))
        xT = biga.tile([P, KB * S], F16, tag="xT")
        wk = biga.tile([P, KB * NX], F16, tag="wk")
        wv = biga.tile([P, KB * NX], F16, tag="wv")
        pt0_pool = stq.enter_context(tc.tile_pool(name="pt0", bufs=9, side="right"))
        pb_pool = stq.enter_context(
            tc.tile_pool(name="pb", bufs=2, space="PSUM", side="right")
        )
        pbq_pool = stq.enter_context(
            tc.tile_pool(name="pbq", bufs=2, space="PSUM", side="right")
        )
        sc0_pool = stq.enter_context(
            tc.tile_pool(name="sc0", bufs=2, space="PSUM", side="right")
        )

        def load_packed(ring, dst, r0, k0, k1):
            # dst[p, k*NX + c] = blob[r0 + 128k + p, c] for k in [k0, k1)
            cols = NX
            src = blob_d[r0 + P * k0 : r0 + P * k1, :].rearrange(
                "(k p) c -> p k c", p=P
            )
            ring.dma_start(
                dst[:, cols * k0 : cols * k1].rearrange("p (k c) -> p k c", c=cols),
                src,
            )

        def load_strip(ring, dst, r0, m):
            # strip m is host-packed as 128 contiguous blob rows
            ring.dma_start(
                dst[:, NX * m : NX * (m + 1)], blob_d[r0 + P * m : r0 + P * (m + 1), :]
            )

        def load_qhalf(ring, t):
            # qT half t is host-packed [128, KB*CW] stored as 512 blob rows
            src = blob_d[R_QT + 4 * P * t : R_QT + 4 * P * (t + 1), :].rearrange(
                "(p q) c -> p q c", p=P
            )
            dst = qT[:, KB * CW * t : KB * CW * (t + 1)].rearrange(
                "p (q c) -> p q c", c=NX
            )
            ring.dma_start(dst, src)

        # ---- DMA issue: both HWDGE rings, strictly in consumption order.
        # The transfer engine is a single shared resource, so the critical
        # prefix (xT+wv for Vproj, then exactly the strips scores(0) needs)
        # goes first; everything else streams under the t0 pass.
        rings = [nc.sync, nc.scalar]
        load_packed(nc.sync, xT, R_XT, 0, 2)      # first x/wv pair, then wk
        load_packed(nc.scalar, wv, R_WV, 0, 2)    # strip 0 (the prologue
        load_strip(nc.sync, wk, R_WK, 0)          # runs kproj(0) k-major
        for k0 in range(2, KB, 2):                # alongside the paced Vproj)
            load_packed(nc.sync, xT, R_XT, k0, k0 + 2)
            load_packed(nc.scalar, wv, R_WV, k0, k0 + 2)
        load_strip(nc.scalar, wq, R_WQ, 0)
        load_qhalf(nc.sync, 0)
        for m in range(1, KB):                    # remaining strips, paced
            load_strip(rings[m % 2], wk, R_WK, m)
            load_strip(rings[(m + 1) % 2], wq, R_WQ, m)
        load_qhalf(nc.scalar, 1)
        load_packed(nc.sync, wp, R_WP, 0, KB)

        cp_alt = [0]

        def psum_copy(dst, src, eng=None):
            if eng is None:
                eng = "v" if cp_alt[0] % 2 == 0 else "s"
                cp_alt[0] += 1
            if eng == "v":
                nc.vector.tensor_copy(dst, src)
            else:
                nc.scalar.copy(dst, src)

        # ones columns of v_aug
        for sb in range(SB):
            va = v_aug[:, sb * H * HC : (sb + 1) * H * HC].rearrange(
                "p (h c) -> p h c", c=HC
            )[:, :, HD : HD + 1]
            nc.vector.tensor_copy(
                va, ones16.rearrange("p (h o) -> p h o", o=1)
            )
        mark("setup")

        def v_copy_out(q, sb, ps, eng=None):
            c0 = NW * q
            if wab:
                nc.tensor.matmul(
                    ps[:, :NW],
                    ones_row[:, :P],
                    b_v[:, c0 : c0 + NW],
                    start=False,
                    stop=True,
                )
            h0 = c0 // HD
            nh = NW // HD
            dst = v_aug[
                :, sb * H * HC + h0 * HC : sb * H * HC + (h0 + nh) * HC
            ].rearrange("p (h c) -> p h c", c=HC)[:, :, :HD]
            srcv = ps[:, :NW].rearrange("p (h c) -> p h c", c=HD)
            psum_copy(dst, srcv, eng)

        def v_quarter_half(q, sbs, eng=None):
            # v columns for heads (2q, 2q+1), row blocks sbs
            c0 = NW * q
            for sb in sbs:
                ps = pb_pool.tile([P, CW], F32, tag="pb")
                for k in range(KB):
                    nc.tensor.matmul(
                        ps[:, :NW],
                        xT[:, S * k + P * sb : S * k + P * (sb + 1)],
                        wv[:, NX * k + c0 : NX * k + c0 + NW],
                        start=(k == 0),
                        stop=(k == KB - 1) and not wab,
                    )
                v_copy_out(q, sb, ps, eng)

        def v_quarter_kmajor(q, sb0, with_k0=False):
            # k-major over a quad of row blocks: four accumulation groups in
            # parallel (borrowing the not-yet-used sc0/at PSUM rings) so the
            # PE keeps pace with the xT/wv chunk stream instead of stalling
            # on one block's full k-chain.  with_k0 additionally interleaves
            # kproj(0)'s two half-tiles (pb ring) into the same k-stream.
            c0 = NW * q
            accs = []
            for i in range(4):
                pool, tg = (sc0_pool, "sc") if i < 2 else (at_pool, "at")
                acc = pool.tile([P, CW], F32, tag=tg, name=f"vacc{i}")
                accs.append(acc)
            accsk = []
            if with_k0:
                for i in range(2):
                    acck = pb_pool.tile([P, CW], F32, tag="pb", name=f"kacc{i}")
                    accsk.append(acck)
            for k in range(KB):
                for i in range(4):
                    sb = sb0 + i
                    nc.tensor.matmul(
                        accs[i][:, :NW],
                        xT[:, S * k + P * sb : S * k + P * (sb + 1)],
                        wv[:, NX * k + c0 : NX * k + c0 + NW],
                        start=(k == 0),
                        stop=(k == KB - 1) and not wab,
                    )
                for t in range(len(accsk)):
                    nc.tensor.matmul(
                        accsk[t][:, :CW],
                        wk[:, P * k : P * (k + 1)],
                        xT[:, S * k + CW * t : S * k + CW * (t + 1)],
                        start=(k == 0),
                        stop=(k == KB - 1) and not wab,
                    )
            for i in range(4):
                v_copy_out(q, sb0 + i, accs[i])
            for t in range(len(accsk)):
                if wab:
                    nc.tensor.matmul(
                        accsk[t][:, :CW],
                        b_k[:, 0:P],
                        ones_row[:, :CW],
                        start=False,
                        stop=True,
                    )
                psum_copy(kT[:, CW * t : CW * (t + 1)], accsk[t][:, :CW])

        def kproj(m, eng=None):
            # kT[:, S*m : S*(m+1)] = (Wk d-block m).T @ xT
            for t in range(2):
                c0, c1 = CW * t, CW * (t + 1)
                ps = pb_pool.tile([P, CW], F32, tag="pb")
                for k in range(KB):
                    nc.tensor.matmul(
                        ps[:, :CW],
                        wk[:, NX * m + P * k : NX * m + P * (k + 1)],
                        xT[:, S * k + c0 : S * k + c1],
                        start=(k == 0),
                        stop=(k == KB - 1) and not wab,
                    )
                if wab:
                    nc.tensor.matmul(
                        ps[:, :CW],
                        b_k[:, P * m : P * (m + 1)],
                        ones_row[:, :CW],
                        start=False,
                        stop=True,
                    )
                psum_copy(kT[:, S * m + c0 : S * m + c1], ps[:, :CW], eng)

        def qproj(t, m, dst, eng=None, pool=None):
            c0, c1 = CW * t, CW * (t + 1)
            pool = pool or pbq_pool
            ps = pool.tile([P, CW], F32, tag=pool.name)
            for k in range(KB):
                nc.tensor.matmul(
                    ps[:, :CW],
                    wq[:, NX * m + P * k : NX * m + P * (k + 1)],
                    qT[:, KB * CW * t + CW * k : KB * CW * t + CW * (k + 1)],
                    start=(k == 0),
                    stop=(k == KB - 1) and not wab,
                )
            if wab:
                nc.tensor.matmul(
                    ps[:, :CW],
                    b_q[:, P * m : P * (m + 1)],
                    ones_row[:, :CW],
                    start=False,
                    stop=True,
                )
            psum_copy(dst, ps[:, :CW], eng)

        def scores_unit(t, m, qsrc, PTa, PTb):
            # odd head first: its exp() stream drains on the scalar engine
            # while the even head's score matmuls run, so attnT (odd first)
            # never waits on exp.  t0: per-j single-bank score tiles + one
            # exp per j; t1: j-pairs share a 2-bank tile + one exp per pair.
            c0, c1 = CW * t, CW * (t + 1)
            jmax = min(SB, (c1 + P - 1) // P)
            for h, PT in ((2 * m + 1, PTb), (2 * m, PTa)):
                po = HD * (h % 2)
                if t == 0:
                    for j in range(jmax):
                        lo = max(c0, P * j)
                        ps = sc0_pool.tile([P, CW], F32, tag="sc")
                        nc.tensor.matmul(
                            ps[:, : c1 - lo],
                            kT[po : po + HD, S * m + P * j : S * m + P * (j + 1)],
                            qsrc[po : po + HD, lo - c0 : c1 - c0],
                            start=True,
                            stop=True,
                        )
                        nc.scalar.activation(
                            PT[:, CW * j + (lo - c0) : CW * j + (c1 - c0)],
                            ps[:, : c1 - lo],
                            EXPF,
                            scale=scale,
                        )
                else:
                    for j0 in range(0, jmax, 2):
                        ps = sc1_pool.tile([P, 2 * CW], F32, tag="sc1")
                        for j in (j0, j0 + 1):
                            lo = max(c0, P * j)
                            nc.tensor.matmul(
                                ps[:, CW * (j - j0) + (lo - c0) : CW * (j - j0) + (c1 - c0)],
                                kT[po : po + HD, S * m + P * j : S * m + P * (j + 1)],
                                qsrc[po : po + HD, lo - c0 : c1 - c0],
                                start=True,
                                stop=True,
                            )
                        # one exp over the pair span; any gap region holds
                        # stale-but-bounded scores and is never read back
                        lo0 = max(c0, P * j0)
                        nc.scalar.activation(
                            PT[:, CW * j0 + (lo0 - c0) : CW * (j0 + 1) + (c1 - c0)],
                            ps[:, lo0 - c0 : CW + (c1 - c0)],
                            EXPF,
                            scale=scale,
                        )
                for j in range(jmax):
                    lo = max(c0, P * j)
                    if lo == P * j:
                        # diagonal block: zero strictly-lower (sk>sq)
                        d0 = CW * j + (lo - c0)
                        nc.vector.tensor_mul(
                            PT[:, d0 : d0 + P], PT[:, d0 : d0 + P], mask
                        )

        def attn_tail(t, m, h, psA):
            c0, c1 = CW * t, CW * (t + 1)
            W = c1 - c0
            po = HD * (h % 2)
            rec = nrm_pool.tile([P, CW], F32, tag="rec")
            nc.vector.reciprocal(rec[HD : HD + 1, :W], psA[HD : HD + 1, :W])
            # hop 1/l to partition 0 (SBUF-to-SBUF DMA crosses partitions;
            # gpsimd broadcast only reads partition 0)
            nc.sync.dma_start(rec[0:1, :W], rec[HD : HD + 1, :W])
            bcs = nrm_pool.tile([P, CW], F32, tag="bcs")
            nc.gpsimd.partition_broadcast(bcs[:HD, :W], rec[0:1, :W])
            if po == 0:
                nc.vector.tensor_mul(
                    aTt[m][t][:HD, :W], psA[:HD, :W], bcs[:HD, :W]
                )
            else:
                ash = nrm_pool.tile([P, CW], F16, tag="ash")
                nc.vector.tensor_mul(ash[:HD, :W], psA[:HD, :W], bcs[:HD, :W])
                # partition shift 0->64 via SBUF-to-SBUF DMA
                nc.sync.dma_start(aTt[m][t][po : po + HD, :W], ash[:HD, :W])

        def attnT_unit(t, m, PTa, PTb):
            # odd head first: its longer normalize tail (partition-shift DMA)
            # overlaps the even head's attnT matmuls + tail.
            c0, c1 = CW * t, CW * (t + 1)
            jmax = min(SB, (c1 + P - 1) // P)
            for h, PT in ((2 * m + 1, PTb), (2 * m, PTa)):
                psA = at_pool.tile([P, CW], F32, tag="at")
                for j in range(jmax):
                    lo = max(c0, P * j)
                    nc.tensor.matmul(
                        psA[:HC, lo - c0 : c1 - c0],
                        v_aug[:, (j * H + h) * HC : (j * H + h + 1) * HC],
                        PT[:, CW * j + (lo - c0) : CW * j + (c1 - c0)],
                        start=(j == 0),
                        stop=(j == jmax - 1),
                    )
                attn_tail(t, m, h, psA)

        # ---- prologue: Vq0 runs k-major, paced by the xT/wv chunk stream;
        # kproj(0) + qproj(0,0) follow as soon as their strips land, so the
        # exp stream starts ~18us in.  The remaining V quarters and kproj
        # strips slide into the t0 units as filler.
        v_quarter_kmajor(0, 0, with_k0=True)
        v_quarter_kmajor(0, 4)
        kproj(1)
        qcur = qTmp_pool.tile([P, CW], F16, tag="qTmp")
        qproj(0, 0, qcur[:, :CW])
        mark("prologue")

        # ---- fused t0 pass (sq 0..CW-1) ----
        for m in range(SB):
            PTa = pt0_pool.tile([P, 4 * CW], F16, tag="pt")
            PTb = pt0_pool.tile([P, 4 * CW], F16, tag="pt")
            scores_unit(0, m, qcur[:, :CW], PTa, PTb)
            # PE filler between scores and attnT hides the exp latency;
            # kproj(m+1) is just-in-time for the next unit's scores
            if 1 <= m <= SB - 2:
                kproj(m + 1)
            if m <= 5:
                v_quarter_half(1 + m // 2, range(4 * (m % 2), 4 * (m % 2) + 4))
            if m < SB - 1:
                qnext = qTmp_pool.tile([P, CW], F16, tag="qTmp")
                qproj(0, m + 1, qnext[:, :CW], "v")
            if m == SB - 1:
                # t1 unit 0's q; the rest are produced inside the t1 pass
                qproj(1, 0, qT1[:, 0:CW], "s")
            attnT_unit(0, m, PTa, PTb)
            if m < SB - 1:
                qcur = qnext
            mark(f"t0m{m}")
        stq.close()  # release xT/wk/wv, pt0, pb, pbq, sc0

        # ---- t1 pass (sq CW..S-1) + c_proj of the t0 rows ----
        sc1_pool = top.enter_context(tc.tile_pool(name="sc1", bufs=2, space="PSUM"))
        pd_pool = top.enter_context(tc.tile_pool(name="pd", bufs=2, space="PSUM"))
        yo_pool = top.enter_context(tc.tile_pool(name="yo", bufs=6))
        yob_pool = top.enter_context(tc.tile_pool(name="yob", bufs=3))
        pt1_pool = top.enter_context(tc.tile_pool(name="pt1", bufs=4))

        def cproj_group(sb, nh, eng=None):
            c0 = CW * nh
            ps = pd_pool.tile([P, CW], F32, tag="pd")
            for k in range(KB):
                nc.tensor.matmul(
                    ps[:, :CW],
                    aTt[k][sb // 4][:, P * (sb % 4) : P * (sb % 4 + 1)],
                    wp[:, NX * k + c0 : NX * k + c0 + CW],
                    start=(k == 0),
                    stop=(k == KB - 1) and not wpb,
                )
            if wpb:
                nc.tensor.matmul(
                    ps[:, :CW],
                    ones_row[:, :P],
                    b_p[:, c0 : c0 + CW],
                    start=False,
                    stop=True,
                )
            yo = yo_pool.tile([P, CW], F16, tag="yo")
            psum_copy(yo[:, :CW], ps[:, :CW], eng)
            out_dma(out_d[P * sb : P * (sb + 1), c0 : c0 + CW], yo[:, :CW])

        for m in range(SB - 1):
            PTa = pt1_pool.tile([P, SB * CW], F16, tag="pt")
            PTb = pt1_pool.tile([P, SB * CW], F16, tag="pt")
            scores_unit(1, m, qT1[:, CW * m : CW * (m + 1)], PTa, PTb)
            if m >= 1:
                g = m - 1
                # nh=0 groups first: they only need the first wp half
                cproj_group(g if g < 4 else g - 4, 0 if g < 4 else 1, "v")
            # next unit's q projection: PE filler that rebalances the
            # otherwise scalar-engine-bound t1 units
            qproj(1, m + 1, qT1[:, CW * (m + 1) : CW * (m + 2)], "v",
                  pool=pd_pool)
            attnT_unit(1, m, PTa, PTb)
            mark(f"t1m{m}")

        # Trailing c_proj for the t1 rows.  The k=7 chunk of aT is the last
        # to normalize (its odd head arrives via the partition-shift DMA), so
        # each group's k=7 matmul is deferred one group: the PE fills the
        # normalize latency with the next group's k=0..6 accumulation.
        def trail_k06(sb, nh):
            # alternate accumulators between the pd ring and the (free
            # after unit 7's scores) sc1 ring, so two trail groups can be
            # in flight while each pool only sees depth-1 pipelining
            if nh == 0:
                ps = pd_pool.tile([P, CW], F32, tag="pd", name="trailpd")
            else:
                ps = sc1_pool.tile([P, 2 * CW], F32, tag="sc1", name="trailsc")
            for k in range(KB - 1):
                nc.tensor.matmul(
                    ps[:, :CW],
                    aTt[k][1][:, P * (sb - 4) : P * (sb - 3)],
                    wp[:, NX * k + CW * nh : NX * k + CW * (nh + 1)],
                    start=(k == 0),
                    stop=False,
                )
            return ps

        def trail_fin(ps, sb, nh):
            k = KB - 1
            nc.tensor.matmul(
                ps[:, :CW],
                aTt[k][1][:, P * (sb - 4) : P * (sb - 3)],
                wp[:, NX * k + CW * nh : NX * k + CW * (nh + 1)],
                start=False,
                stop=not wpb,
            )
            if wpb:
                nc.tensor.matmul(
                    ps[:, :CW],
                    ones_row[:, :P],
                    b_p[:, CW * nh : CW * (nh + 1)],
                    start=False,
                    stop=True,
                )
            if nh == 0:
                yob_cur[0] = yob_pool.tile([P, NX], F16, tag="yob", name="yob")
            yob = yob_cur[0]
            psum_copy(yob[:, CW * nh : CW * (nh + 1)], ps[:, :CW],
                      "v" if nh == 0 else "s")
            if nh == 1:
                # both halves staged: one [128, NX] DMA per row block halves
                # the trail's descriptor-generator load and lets the last
                # transfer start as soon as the two (parallel-engine) copies
                # land
                out_dma(out_d[P * sb : P * (sb + 1), :], yob[:, :])

        # Last t1 unit with the trail's k=0..6 accumulations (which only
        # need units 0..6) interleaved — including between its two attnT
        # heads — so the trail pipeline is already two groups deep when
        # unit 7's normalize lands.
        PTa = pt1_pool.tile([P, SB * CW], F16, tag="pt")
        PTb = pt1_pool.tile([P, SB * CW], F16, tag="pt")
        m7 = SB - 1
        scores_unit(1, m7, qT1[:, CW * m7 : CW * SB], PTa, PTb)
        cproj_group(2, 1, "v")
        pend = (trail_k06(4, 0), 4, 0)
        c0, c1 = CW, 2 * CW
        for h, PT in ((2 * m7 + 1, PTb), (2 * m7, PTa)):
            psA = at_pool.tile([P, CW], F32, tag="at", name="psA7")
            for j in range(SB):
                lo = max(c0, P * j)
                nc.tensor.matmul(
                    psA[:HC, lo - c0 : c1 - c0],
                    v_aug[:, (j * H + h) * HC : (j * H + h + 1) * HC],
                    PT[:, CW * j + (lo - c0) : CW * j + (c1 - c0)],
                    start=(j == 0),
                    stop=(j == SB - 1),
                )
            if h % 2 == 1:
                pend2 = (trail_k06(4, 1), 4, 1)
            attn_tail(1, m7, h, psA)
        mark(f"t1m{m7}")
        cproj_group(3, 1, "v")
        stq_b.close()  # release qT, wq, qTmp ring

        for sb in range(4, SB):
            for nh in range(2):
                if sb == 4:
                    continue
                ps = trail_k06(sb, nh)
                trail_fin(*pend)
                pend = pend2
                pend2 = (ps, sb, nh)
        trail_fin(*pend)
        trail_fin(*pend2)
        mark("tail")

    nc.compile()
    return nc


def get_module(S, NX, H, with_attn_bias, with_proj_bias, n_cores=8):
    key = (S, NX, H, with_attn_bias, with_proj_bias, n_cores)
    if key not in _CACHE:
        _CACHE[key] = build_module(
            S, NX, H, with_attn_bias, with_proj_bias, n_cores
        )
    return _CACHE[key]


def _strips(w):
    # [NX, NX] -> strip-major rows: strip m = w[:, 128m:128(m+1)] packed as
    # [128, KB*128] and stored as 128 rows
    NX = w.shape[0]
    KB = NX // P
    return np.ascontiguousarray(
        w.reshape(KB, P, KB, P).transpose(2, 1, 0, 3).reshape(NX, NX)
    )


def _qhalves(qt, CW=512):
    # qT [NX, S] -> half-major: half t = qt[:, CW*t:CW*(t+1)] packed as
    # [128, KB*CW] and stored as 4*128 rows of NX
    NX, S = qt.shape
    KB = NX // P
    out = np.empty((NX, S), dtype=qt.dtype)
    for t in range(S // CW):
        half = qt[:, CW * t : CW * (t + 1)]           # [NX, CW]
        packed = half.reshape(KB, P, CW).transpose(1, 0, 2).reshape(P, KB * CW)
        out[4 * P * t : 4 * P * (t + 1)] = packed.reshape(4 * P, NX)
    return out


def make_in_maps(x, query, c_attn_w, c_attn_b, c_proj_w, c_proj_b, n_cores=8):
    """Pack the per-core fp16 blob inputs: [xT | qT | Wk | Wv | Wq | Wp |
    consts | bias].  Wk/Wq are strip-major, qT is half-major (see
    build_module's loaders)."""
    B, S, NX = x.shape
    H = 16
    R_TOT = 6 * NX + P + 5
    base = np.zeros((R_TOT, NX), dtype=np.float16)
    base[2 * NX : 3 * NX] = _strips(
        c_attn_w[:, NX : 2 * NX].astype(np.float16)
    )                                                         # Wk strips
    base[3 * NX : 4 * NX] = c_attn_w[:, 2 * NX : 3 * NX]      # Wv
    base[4 * NX : 5 * NX] = _strips(
        c_attn_w[:, :NX].astype(np.float16)
    )                                                         # Wq strips
    base[5 * NX : 6 * NX] = c_proj_w                          # Wp
    R_CONST = 6 * NX
    base[R_CONST : R_CONST + P, :P] = np.triu(np.ones((P, P), dtype=np.float16))
    base[R_CONST : R_CONST + P, P : P + H] = 1.0
    R_BIAS = 6 * NX + P
    base[R_BIAS] = 1.0                                        # ones row
    base[R_BIAS + 1, :NX] = c_attn_b[:NX]
    base[R_BIAS + 2, :NX] = c_attn_b[NX : 2 * NX]
    base[R_BIAS + 3, :NX] = c_attn_b[2 * NX :]
    base[R_BIAS + 4, :NX] = c_proj_b

    blobs = np.empty((n_cores, R_TOT, NX), dtype=np.float16)
    for c in range(n_cores):
        blobs[c] = base
        blobs[c, :NX] = x[c % B].T
        blobs[c, NX : 2 * NX] = _qhalves(
            query[c % B].T.astype(np.float16)
        )
    return [{"blob": blobs[c]} for c in range(n_cores)]


def kernel(x, query, c_attn_w, c_attn_b, c_proj_w, c_proj_b, _trace=False):
    x = np.asarray(x, dtype=np.float32)
    query = np.asarray(query, dtype=np.float32)
    c_attn_w = np.asarray(c_attn_w, dtype=np.float32)
    c_attn_b = np.asarray(c_attn_b, dtype=np.float32)
    c_proj_w = np.asarray(c_proj_w, dtype=np.float32)
    c_proj_b = np.asarray(c_proj_b, dtype=np.float32)

    B, S, NX = x.shape
    H = 16
    wab = bool(np.any(c_attn_b != 0))
    wpb = bool(np.any(c_proj_b != 0))
    n_cores = 8
    nc = get_module(S, NX, H, wab, wpb, n_cores)

    in_maps = make_in_maps(
        x, query, c_attn_w, c_attn_b, c_proj_w, c_proj_b, n_cores
    )
    res = run_bass_kernel_spmd(
        nc, in_maps, core_ids=list(range(n_cores)), trace=_trace
    )
    out = np.stack(
        [res.results[c]["out"].astype(np.float32) for c in range(B)], axis=0
    )
    if _trace:
        kernel._last_results = res
    return out
